# revision 30
# baseline (speedup 1.0000x reference)
"""GCN graph classifier on 8 Trainium2 NeuronCores (Bass/Tile SPMD).

Strategy (v2):
  - Nodes bin-packed into 400 balanced tiles (8 cores x 50 tiles x 128 slots)
    with ~equal incident-edge count per tile -> uniform SPMD program, NCH
    gather chunks per tile.
  - norm = dinv[src]*dinv[dst] is separable: H' = dinv * (h @ W) is computed
    locally and all-gathered (fp16); the dinv[dst] factor is baked into the
    host-precomputed segment-sum selection matrices S.
  - S matrices (one-hot columns scaled by dinv[dst]) are STATIC graph
    structure: built dense fp16 on the host once, streamed from DRAM by the
    Sync engine each layer.  This keeps the DVE out of the per-chunk path
    entirely (was the #1 bottleneck).
  - Self-loops are not gathered: each tile gets a "self chunk"
    psum += hp_tile^T @ diag((1+k)*dinv) using the locally-resident H' tile
    (k = count of natural i->i edges).  Cuts ~8% of gather indices.
  - Per layer: local fp16 matmul -> AllGather H' -> dma_gather neighbor rows
    (int16 indices, two DRAM banks with an overlap window, 3840-idx calls on
    a 64KB descriptor carveout) -> PE accumulates
    psum[f,d] += gathered[e,f]^T S[e,d] -> ACT relu(psum + b) -> fp16 hT.
  - Mean-pool partials ([64,128] per core) via host-built one-hot(batch)
    matmul; host sums partials, divides by counts, applies final linear.
"""
import numpy as np

N = 50000
E = 600000
P = 128
G = 64
NCORES = 8
TPC = 50                 # tiles per core
NTILES = NCORES * TPC    # 400
SHARD = TPC * P          # 6400 rows per core
ROWS = NCORES * SHARD    # 51200
BANKLO_END = 32768
BANKHI_START = ROWS - 32768   # 18432
SCRATCH = 65536          # SWDGE descriptor carveout (4096 descs/queue ring)
RINGCAP = SCRATCH // 16 - 64  # usable descriptors per gather call

_PROG_CACHE = {}
LAST_RESULT = None


def _prepare(x, edge_index, batch, dinv):
    """Host-side graph partitioning and metadata packing."""
    src_all = edge_index[0]
    dst_all = edge_index[1]
    nonself = src_all != dst_all
    src = src_all[nonself]
    dst = dst_all[nonself]
    # count of natural self edges per node (folded into the self chunk)
    selfcnt = np.bincount(dst_all[~nonself], minlength=N)

    indeg = np.bincount(dst, minlength=N)   # non-self in-degree (load balance)

    # ---- bin-pack nodes into NTILES tiles of <=128 slots, balancing indeg
    import heapq
    order = np.argsort(-indeg, kind="stable")
    heap = [(0, t) for t in range(NTILES)]
    heapq.heapify(heap)
    tile_of = np.empty(N, np.int32)
    slot_of = np.empty(N, np.int32)
    counts = np.zeros(NTILES, np.int32)
    loads = np.zeros(NTILES, np.int64)
    for n in order:
        while True:
            load, t = heapq.heappop(heap)
            if counts[t] < P:
                break
        tile_of[n] = t
        slot_of[n] = counts[t]
        counts[t] += 1
        loads[t] = load + indeg[n]
        if counts[t] < P:
            heapq.heappush(heap, (loads[t], t))
    # hfull row layout [half, core, tile-in-half, slot]: each half-AllGather
    # output is a contiguous slice, so the first half can be dispatched
    # mid-layer and overlap compute.
    HT = TPC // 2
    core_np = (tile_of // TPC).astype(np.int64)
    tl_np = (tile_of % TPC).astype(np.int64)
    half_np = (tl_np >= HT).astype(np.int64)
    pos = (half_np * (NCORES * HT * P) + core_np * (HT * P) +
           (tl_np - half_np * HT) * P + slot_of)

    # ---- per-edge quantities
    epos = pos[src]                       # source position in Hfull
    etile = tile_of[dst]                  # destination tile
    eslot = slot_of[dst].astype(np.int64)
    esval = dinv[dst].astype(np.float32)
    rigid_lo = epos < BANKHI_START
    rigid_hi = epos >= BANKLO_END
    # sort edges by (tile, bankclass): 0=rigid_lo, 1=flex, 2=rigid_hi
    bclass = np.ones(len(epos), np.int8)
    bclass[rigid_lo] = 0
    bclass[rigid_hi] = 2
    eorder = np.lexsort((bclass, etile))
    epos, etile, eslot, esval, bclass = (
        epos[eorder], etile[eorder], eslot[eorder], esval[eorder], bclass[eorder])
    tile_start = np.searchsorted(etile, np.arange(NTILES + 1))

    n_all = np.diff(tile_start)
    n_lo = np.zeros(NTILES, np.int64)
    n_hi = np.zeros(NTILES, np.int64)
    np.add.at(n_lo, etile[bclass == 0], 1)
    np.add.at(n_hi, etile[bclass == 2], 1)
    nchlo_min = int(np.ceil(n_lo.max() / P))
    nchhi_min = int(np.ceil(n_hi.max() / P))
    nch_min = int(np.ceil(n_all.max() / P))
    NCH = max(nchlo_min + nchhi_min, nch_min)
    NCHLO = nchlo_min + (NCH - nchlo_min - nchhi_min) // 2
    NCHHI = NCH - NCHLO
    assert NCHLO * P >= n_lo.max() and NCHHI * P >= n_hi.max()
    CPT = 1 + NCH            # chunks per tile incl self chunk
    # largest call-group size with TWO calls in the descriptor ring (so
    # descriptor generation can run ahead of the DMA drain per queue)
    GRP = max(g for g in (10, 5, 2, 1)
              if 2 * g * max(NCHLO, NCHHI) * P <= RINGCAP)
    NGRP = TPC // GRP

    # ---- pack per-core metadata
    # idx layout per core: [NGRP groups][lo call: GRP*NCHLO*128][hi call: ...]
    callcols = GRP * NCH * 8          # int16 columns per group (lo+hi calls)
    idx16 = np.zeros((NCORES, 16, NGRP * callcols), np.int16)
    svals = np.zeros((NCORES, TPC * CPT, P, P), np.float16)
    for t in range(NTILES):
        c, tl = divmod(t, TPC)
        s, e = tile_start[t], tile_start[t + 1]
        ep, es, ev, b = epos[s:e], eslot[s:e], esval[s:e], bclass[s:e]
        ndeg = e - s
        lo_cnt = int((b == 0).sum())
        flex_cnt = int((b == 1).sum())
        need_lo = max(lo_cnt, ndeg - NCHHI * P)
        take_flex = min(flex_cnt, max(0, min(NCHLO * P, need_lo + flex_cnt) - lo_cnt))
        nlo = lo_cnt + take_flex
        assert nlo <= NCHLO * P and (ndeg - nlo) <= NCHHI * P, (t, ndeg, nlo)
        g, tau = divmod(tl, GRP)
        callbase = g * callcols
        iolo = callbase + tau * NCHLO * 8
        iohi = callbase + GRP * NCHLO * 8 + tau * NCHHI * 8
        ilo = np.arange(nlo)
        idx16[c, ilo % 16, iolo + ilo // 16] = ep[:nlo]
        ihi = np.arange(ndeg - nlo)
        idx16[c, ihi % 16, iohi + ihi // 16] = ep[nlo:] - BANKHI_START
        # S chunks: tile-major [self, lo0..lo(NCHLO-1), hi0..hi(NCHHI-1)]
        chbase = tl * CPT + 1
        svals[c, chbase + ilo // P, ilo % P, es[:nlo]] = ev[:nlo]
        svals[c, chbase + NCHLO + ihi // P, ihi % P, es[nlo:]] = ev[nlo:]
    idx16 = np.tile(idx16, (1, 8, 1))  # replicate across 8 gpsimd q7 cores

    # ---- per-core node data
    xT_local = np.zeros((NCORES, P, SHARD), np.float16)
    dinv_col = np.ones((NCORES, P, TPC), np.float32)
    oh = np.zeros((NCORES, P, TPC * G), np.float16)
    core_of = tile_of // TPC
    tl_of = tile_of % TPC
    row_in_shard = tl_of * P + slot_of
    # self chunk diagonal value: (1 + selfcnt[n]) * dinv[n]; hp already
    # carries one dinv factor, contribution = (1+k) * dinv^2 * (hW)[n].
    selfval = (1.0 + selfcnt) * dinv
    for c in range(NCORES):
        m = core_of == c
        xT_local[c][:, row_in_shard[m]] = x[m].astype(np.float16).T
        dinv_col[c][slot_of[m], tl_of[m]] = dinv[m]
        oh[c][slot_of[m], tl_of[m] * G + batch[m]] = 1.0
        # self chunk: svals[c, tl*CPT, d, d] = selfval
        svals[c, tl_of[m] * CPT, slot_of[m], slot_of[m]] = selfval[m]

    # S DRAM layout: [128 partitions = chunk row e, free = (chunk, d)]
    s_stream = np.ascontiguousarray(
        svals.transpose(0, 2, 1, 3).reshape(NCORES, P, TPC * CPT * P))

    return dict(NCH=NCH, NCHLO=NCHLO, NCHHI=NCHHI, GRP=GRP, idx16=idx16,
                s_stream=s_stream, xT_local=xT_local, dinv_col=dinv_col, oh=oh)


def _build_program(NCH, NCHLO, NCHHI, GRP):
    NGRP = TPC // GRP
    import concourse.bacc as bacc
    import concourse.mybir as mybir
    from concourse.tile import TileContext
    from concourse.library_config import mlp

    f32 = mybir.dt.float32
    f16 = mybir.dt.float16
    CPT = 1 + NCH
    nc = bacc.Bacc("TRN2", target_bir_lowering=False, debug=False,
                   num_devices=NCORES, num_swdge_queues=4,
                   dynamic_dma_scratch_size=SCRATCH)
    xT_in = nc.declare_dram_parameter("xT", [P, SHARD], f16, isOutput=False)
    idx_in = nc.declare_dram_parameter("idx", [P, NGRP * GRP * NCH * 8],
                                       mybir.dt.int16, isOutput=False)
    s_in = nc.declare_dram_parameter("sstream", [P, TPC * CPT * P], f16,
                                     isOutput=False)
    dc_in = nc.declare_dram_parameter("dinvc", [P, TPC], f32, isOutput=False)
    oh_in = nc.declare_dram_parameter("oh", [P, TPC * G], f16, isOutput=False)
    id_in = nc.declare_dram_parameter("identf16", [P, P], f16, isOutput=False)
    w_in = [nc.declare_dram_parameter(f"W{l}", [P, P], f16, isOutput=False)
            for l in range(3)]
    b_in = nc.declare_dram_parameter("bias", [P, 3], f32, isOutput=False)
    pool_out = nc.declare_dram_parameter("pool", [G, P], f32, isOutput=True)

    shard_d = [nc.dram_tensor(f"shard{l}", [SHARD, P], f16) for l in range(3)]
    hfull_d = [nc.dram_tensor(f"hfull{l}", [ROWS, P], f16, addr_space="Shared")
               for l in range(3)]
    rg = [list(range(NCORES))]

    with TileContext(nc) as tc:
        nc.gpsimd.load_library(mlp)
        with tc.tile_pool(name="const", bufs=1) as cpool, \
             tc.tile_pool(name="big", bufs=1) as bigpool, \
             tc.tile_pool(name="gb", bufs=8) as gbpool, \
             tc.tile_pool(name="s", bufs=6) as spool, \
             tc.tile_pool(name="misc", bufs=3) as mpool, \
             tc.tile_pool(name="ps", bufs=2, space="PSUM") as pspool, \
             tc.tile_pool(name="pagg", bufs=3, space="PSUM") as paggpool, \
             tc.tile_pool(name="ppool", bufs=1, space="PSUM") as ppoolpool:
            idxs = cpool.tile([P, NGRP * GRP * NCH * 8], mybir.dt.int16)
            xT = cpool.tile([P, SHARD], f16)
            dc = cpool.tile([P, TPC], f32)
            ohc = cpool.tile([P, TPC * G], f16)
            ident = cpool.tile([P, P], f16)
            wt = [cpool.tile([P, P], f16, name=f"wt{i}") for i in range(3)]
            bias = cpool.tile([P, 3], f32)
            for dst_t, src_t in [(idxs, idx_in), (xT, xT_in), (dc, dc_in),
                                 (ohc, oh_in), (ident, id_in),
                                 (wt[0], w_in[0]), (wt[1], w_in[1]),
                                 (wt[2], w_in[2]), (bias, b_in)]:
                nc.sync.dma_start(out=dst_t[:], in_=src_t[:])

            hT = bigpool.tile([P, TPC * P], f16)   # current layer h^T tiles
            hp = bigpool.tile([P, TPC * P], f16)   # H' (node-part, feat-free)
            pspl = ppoolpool.tile([G, P], f32, space="PSUM")
            dma_sems = [nc.alloc_semaphore(f"gsem{q}") for q in range(4)]
            NPREP = 0                # prep-ahead disabled (Tile books prepped
                                     # gather completion on the wrong sem)
            HT = TPC // 2            # tiles per shard half

            def emit_shard_half(layer, h):
                src = hp[:, h * HT * P:(h + 1) * HT * P].rearrange(
                    "p (t f) -> p t f", t=HT)
                dst = shard_d[layer][h * HT * P:(h + 1) * HT * P, :].rearrange(
                    "(t p) f -> p t f", p=P)
                nc.sync.dma_start(out=dst, in_=src)
                HROWS = NCORES * HT * P
                nc.gpsimd.collective_compute(
                    "AllGather", mybir.AluOpType.bypass, replica_groups=rg,
                    ins=[shard_d[layer][h * HT * P:(h + 1) * HT * P, :]],
                    outs=[hfull_d[layer][h * HROWS:(h + 1) * HROWS, :]])

            def emit_gather(layer, g, gbt, prepare):
                hfull = hfull_d[layer]
                cb = g * (GRP * NCH * 8)
                nlo8 = GRP * NCHLO * 8
                qlo, qhi = (2 * g) % 4, (2 * g + 1) % 4
                klo = dict(prepare_only=True, sem=dma_sems[qlo]) if prepare else {}
                khi = dict(prepare_only=True, sem=dma_sems[qhi]) if prepare else {}
                nc.gpsimd.dma_gather(
                    gbt[:, :GRP * NCHLO, :], hfull[:BANKLO_END, :],
                    idxs[:, cb:cb + nlo8],
                    GRP * NCHLO * P, GRP * NCHLO * P, P,
                    queue_num=qlo, single_packet=False, **klo)
                nc.gpsimd.dma_gather(
                    gbt[:, GRP * NCHLO:, :], hfull[BANKHI_START:, :],
                    idxs[:, cb + nlo8:cb + GRP * NCH * 8],
                    GRP * NCHHI * P, GRP * NCHHI * P, P,
                    queue_num=qhi, single_packet=False, **khi)

            def emit_tile(layer, g, tau, gbt):
                """Aggregation matmuls + relu for one tile; fused next-layer
                H'=dinv*(h@W) (layers 0,1) or pooling (layer 2)."""
                t = g * GRP + tau
                tc0, tc1 = t * P, (t + 1) * P
                st = spool.tile([P, CPT * P], f16)
                nc.sync.dma_start(
                    out=st[:], in_=s_in[:, t * CPT * P:(t + 1) * CPT * P])
                psum = paggpool.tile([P, P], f32, space="PSUM")
                nc.tensor.matmul(out=psum[:], lhsT=hp[:, tc0:tc1],
                                 rhs=st[:, 0:P], start=True, stop=False)
                if tau == 0 and g < NPREP:
                    # Tile books a prepped gather's DMA completion on a DMASW
                    # lane, but the hardware increments the sem baked into
                    # the descriptors (dma_sems) — wait on that explicitly.
                    tgt = 32 * layer + 16 * (g // 2 + 1)
                    nc.tensor.wait_ge(dma_sems[(2 * g) % 4], tgt)
                    nc.tensor.wait_ge(dma_sems[(2 * g + 1) % 4], tgt)
                for c in range(NCH):
                    if c < NCHLO:
                        col = tau * NCHLO + c
                    else:
                        col = GRP * NCHLO + tau * NCHHI + (c - NCHLO)
                    nc.tensor.matmul(
                        out=psum[:], lhsT=gbt[:, col, :],
                        rhs=st[:, (1 + c) * P:(2 + c) * P],
                        start=False, stop=(c == NCH - 1))
                nc.scalar.activation(
                    out=hT[:, tc0:tc1], in_=psum[:],
                    func=mybir.ActivationFunctionType.Relu,
                    bias=bias[:, layer:layer + 1])
                if layer < 2:
                    psH = pspool.tile([P, P], f32, space="PSUM")
                    nc.tensor.matmul(out=psH[:], lhsT=hT[:, tc0:tc1],
                                     rhs=wt[layer + 1][:], start=True, stop=True)
                    nc.vector.tensor_scalar_mul(hp[:, tc0:tc1], psH[:],
                                                dc[:, t:t + 1])
                else:
                    pst = pspool.tile([P, P], f16, space="PSUM")
                    nc.tensor.transpose(out=pst[:], in_=hT[:, tc0:tc1],
                                        identity=ident[:])
                    h3 = mpool.tile([P, P], f16)
                    nc.vector.tensor_copy(out=h3[:], in_=pst[:])
                    nc.tensor.matmul(out=pspl[:],
                                     lhsT=ohc[:, t * G:(t + 1) * G],
                                     rhs=h3[:], start=(t == 0),
                                     stop=(t == TPC - 1))

            def emit_preps(layer):
                """Desc-prep the first NPREP groups of `layer`; the hfull
                read is deferred to the per-queue trigger (Tile moves the
                RAW dep), so the Q7 generates descriptors while preceding
                work and the AllGather run."""
                gbts = []
                for g in range(NPREP):
                    gbt = gbpool.tile([P, GRP * NCH, P], f16)
                    emit_gather(layer, g, gbt, prepare=True)
                    gbts.append(gbt)
                return gbts

            # ---- layer 0 input projection: hp = dinv * (x @ W1)
            # (preps are emitted BEFORE hfull's writer exists so they carry
            # no hfull dep; the AllGather->trigger ordering is forced below
            # via signals_writable on trigger_dma.)
            gbts = emit_preps(0)
            for t in range(TPC):
                tc0, tc1 = t * P, (t + 1) * P
                psH = pspool.tile([P, P], f32, space="PSUM")
                nc.tensor.matmul(out=psH[:], lhsT=xT[:, tc0:tc1], rhs=wt[0][:],
                                 start=True, stop=True)
                nc.vector.tensor_scalar_mul(hp[:, tc0:tc1], psH[:],
                                            dc[:, t:t + 1])
                if t == HT - 1:
                    emit_shard_half(0, 0)
            emit_shard_half(0, 1)

            # heuristic emission point for the first-half collective of the
            # next layer: the aggregation matmuls trail the gather stream by
            # roughly the DMA drain latency (~2 groups).
            K1 = min(NGRP - 2, HT // GRP + 2)

            for layer in range(3):
                if NPREP:
                    # signals_writable makes each trigger a writer of hfull,
                    # so Tile orders it after the AllGather (the prepped DMAs
                    # read hfull only when the trigger fires).
                    for q in range(4):
                        nc.gpsimd.trigger_dma(
                            count=None, queue_num=q,
                            signals_writable=[hfull_d[layer][q:q + 1, :]])
                for g in range(NGRP):
                    if g < NPREP:
                        gbt = gbts[g]
                    else:
                        gbt = gbpool.tile([P, GRP * NCH, P], f16)
                        emit_gather(layer, g, gbt, prepare=False)
                    for tau in range(GRP):
                        emit_tile(layer, g, tau, gbt)
                    if layer < 2 and g == K1:
                        emit_shard_half(layer + 1, 0)
                if layer < 2:
                    gbts = emit_preps(layer + 1)
                    emit_shard_half(layer + 1, 1)

            po = mpool.tile([G, P], f32)
            nc.vector.tensor_copy(out=po[:], in_=pspl[:])
            nc.sync.dma_start(out=pool_out[:], in_=po[:])

    nc.compile()
    return nc


def _install_ntff_shim():
    """Provide antenv.axon_hooks (missing on this image) so trace=True works."""
    import sys
    import types
    try:
        import antenv.axon_hooks  # noqa: F401
        return
    except ImportError:
        pass
    hook = None
    try:
        from trn_agent_boot import trn_boot
        hook = trn_boot._ntff_profile_via_ctypes("/opt/axon/libaxon_pjrt.so")
    except Exception:
        pass
    mod = types.ModuleType("antenv.axon_hooks")
    mod._hook = hook
    mod.get_axon_ntff_profile_hook = lambda: mod._hook
    mod.set_axon_ntff_profile_hook = lambda h: setattr(mod, "_hook", h)
    sys.modules["antenv.axon_hooks"] = mod
    import antenv
    antenv.axon_hooks = mod


def kernel(x, edge_index, batch, W1, b1, W2, b2, W3, b3, Wlin, blin):
    global LAST_RESULT
    from concourse.bass_utils import run_bass_kernel_spmd
    import os

    x = np.asarray(x, np.float32)
    edge_index = np.asarray(edge_index, np.int64)
    batch = np.asarray(batch, np.int64)
    W1, b1, W2, b2, W3, b3 = (np.asarray(a, np.float32)
                              for a in (W1, b1, W2, b2, W3, b3))
    Wlin = np.asarray(Wlin, np.float32)
    blin = np.asarray(blin, np.float32)

    deg = np.bincount(
        np.concatenate([edge_index[1], np.arange(N)]), minlength=N
    ).astype(np.float32)
    dinv = np.where(deg > 0, 1.0 / np.sqrt(deg), 0.0).astype(np.float32)

    meta = _prepare(x, edge_index, batch, dinv)
    NCH, NCHLO, NCHHI = meta["NCH"], meta["NCHLO"], meta["NCHHI"]
    key = (NCH, NCHLO, NCHHI, meta["GRP"])
    if key not in _PROG_CACHE:
        _PROG_CACHE[key] = _build_program(NCH, NCHLO, NCHHI, meta["GRP"])
    nc = _PROG_CACHE[key]

    ident_np = np.eye(P, dtype=np.float16)
    bias_np = np.stack([b1, b2, b3], axis=1).astype(np.float32)  # [128,3]
    in_maps = []
    for c in range(NCORES):
        in_maps.append({
            "xT": meta["xT_local"][c], "idx": meta["idx16"][c],
            "sstream": meta["s_stream"][c], "dinvc": meta["dinv_col"][c],
            "oh": meta["oh"][c], "identf16": ident_np,
            "W0": W1.astype(np.float16), "W1": W2.astype(np.float16),
            "W2": W3.astype(np.float16), "bias": bias_np,
        })
    trace = bool(os.environ.get("BASS_TRACE"))
    if trace:
        _install_ntff_shim()
    try:
        res = run_bass_kernel_spmd(nc, in_maps, list(range(NCORES)), trace=trace)
    except Exception:
        if not trace:
            raise
        os.environ["BASS_NEVER_TRACE"] = "1"
        try:
            res = run_bass_kernel_spmd(nc, in_maps, list(range(NCORES)), trace=False)
        finally:
            os.environ.pop("BASS_NEVER_TRACE", None)
    LAST_RESULT = res

    pool_sum = np.zeros((G, P), np.float64)
    for c in range(NCORES):
        pool_sum += res.results[c]["pool"].astype(np.float64)
    cnt = np.bincount(batch, minlength=G).astype(np.float32)
    pooled = (pool_sum.astype(np.float32)) / np.maximum(cnt, 1.0)[:, None]
    return (pooled @ Wlin + blin).astype(np.float32)
